# revision 32
# baseline (speedup 1.0000x reference)
"""Per-batch (block-diagonal) cross-attention kernel for Trainium2.

Each query row attends only to key/value rows with the same batch id
(ids in [0, 8), both coor arrays sorted). Batch b -> core b: every core
runs one dense attention block of ~1k queries x ~1k keys, C=64, no
collectives.

v2 design (all sizes per core; P=128, Qv = valid queries, Kp = padded
keys, nk = Kp/128):

  Host-side layout (bf16 operands, fp32 accumulate on device):
    - qkT [64, Qp+Kp]  : [Q^T | K^T], bf16, zero-padded
    - kv  [128, nk*65] : KV rows interleaved per k-tile, col 64 = 1.0
                         on valid rows (softmax denominator trick), bf16
  Device per k-tile kti:
    - S^T[k, q] = kT_tile^T @ qT   (PE, bf16, fp32 PSUM)
    - P^T = exp(S^T / 8): split between ACT (exact exp activation) and
      DVE (Schraudolph: i16 = S*A + B, bitcast to bf16 ~= exp, ~1.5%
      per-element, cancels in softmax normalization) so neither engine
      is the wall. No max-subtraction (scores O(1) for randn inputs;
      softmax is shift-invariant).
    - out^T[c, q] += kv_tile^T @ P^T_tile  (PE, stationary = kv tile so
      ldweights is 65 cols and fully hidden; accumulated over kti in
      PSUM). Row 64 = softmax denominator.
  Tail: q columns beyond the PSUM-bank-aligned main region [0:M] are
  computed into a single batched tail PSUM tile (one exp instruction
  for all k-tiles' tails) when they fit one bank, else per-k-tile.
  Output: out^T [65, Qv] fp32 DMA'd straight from PSUM; the host does
  the numerator/denominator divide and the transpose back to [q, c].

  PSUM budget (8 banks): S-main 2 bufs x 2 banks + S-tail 1 bank +
  out^T 3 banks = 8.

  PV passes run interleaved two k-tiles behind S so the PE never waits
  on ACT/DVE exp, and warmup matmuls during the input-DMA head push the
  PE out of its low p-state before real work arrives.
"""

import os
from contextlib import ExitStack

import numpy as np

import concourse.bacc as bacc
import concourse.bass as bass
import concourse.mybir as mybir
import concourse.tile as tile
from concourse.bass_utils import run_bass_kernel_spmd

N_CORES = 8
C = 64
P = 128
KW = C + 1  # kv tile width (65: values + ones column)
SCALE = 1.0 / 8.0  # 1/sqrt(C)
F32 = mybir.dt.float32
BF16 = mybir.dt.bfloat16
I16 = mybir.dt.int16

# Schraudolph exp approximation in bf16: exp(s/8) ~= bitcast_bf16(
# int16(s * A + B)). A folds the 1/sqrt(C) score scale into 2^7/ln2.
SCH_A = 184.66496736312366 / 8.0
SCH_B = 16256.0 - 7.42

# How many of the main exp tiles go to ACT (exact); the rest go to DVE
# (Schraudolph). Tails go to DVE/ACT per layout below.
ACT_MAIN = int(os.environ.get("XATTN_ACT_MAIN", "5"))

BANK_F32 = 512  # fp32 elements per PSUM bank (2KB)

_LAST_RUN = {}


def _round_up(x: int, m: int) -> int:
    return -(-x // m) * m


def _emit2(ctx: ExitStack, tc: "tile.TileContext", out_ap, qkt_ap, kv_ap,
           Qp: int, Kp: int, Qv: int):
    """Actual emitter (see _emit docstring for the design)."""
    nc = tc.nc
    nk = Kp // P

    M = min(Qv, 2 * BANK_F32)
    TW = Qv - M
    tails_batched = TW > 0 and nk * TW <= BANK_F32

    m_chunks = [(0, min(M, BANK_F32))]
    if M > BANK_F32:
        m_chunks.append((BANK_F32, M - BANK_F32))
    o_chunks = [(c, min(BANK_F32, Qv - c)) for c in range(0, Qv, BANK_F32)]

    big = ctx.enter_context(tc.tile_pool(name="big", bufs=1))
    psum_s = ctx.enter_context(tc.tile_pool(name="pss", bufs=2, space="PSUM"))
    psum_o = ctx.enter_context(tc.tile_pool(name="pso", bufs=1, space="PSUM"))
    if TW > 0:
        psum_t = ctx.enter_context(
            tc.tile_pool(name="pst", bufs=1, space="PSUM"))

    qkt = big.tile([C, Qp + Kp], BF16, tag="qkt", name="qkt")
    kv_all = big.tile([P, nk * KW], BF16, tag="kv_all", name="kv_all")
    warm = big.tile([C, 256], BF16, tag="warm", name="warm")

    nc.gpsimd.memset(warm[:], 0.0)

    # Ring layout: the scalar ring carries what slot 0 needs first
    # (qt-c0 then qt-tail, both small); the sync ring carries kT-head,
    # then qt-c1 (needed by S(0) chunk 2), then the rest of kT (first
    # needed at slot 2). kv goes via gpsimd SWDGE in parallel.
    c0w = min(BANK_F32, M)
    nc.sync.dma_start(qkt[:, Qp:Qp + 2 * P], qkt_ap[:, Qp:Qp + 2 * P])
    nc.scalar.dma_start(qkt[:, 0:c0w], qkt_ap[:, 0:c0w])
    if M > c0w:
        nc.sync.dma_start(qkt[:, c0w:M], qkt_ap[:, c0w:M])
    if M < Qp:
        nc.scalar.dma_start(qkt[:, M:Qp], qkt_ap[:, M:Qp])
    nc.sync.dma_start(qkt[:, Qp + 2 * P:Qp + Kp], qkt_ap[:, Qp + 2 * P:Qp + Kp])
    nc.gpsimd.dma_start(kv_all[:], kv_ap[:, :])

    qt = qkt[:, 0:Qp]
    kt = qkt[:, Qp:Qp + Kp]

    pt_t = [big.tile([P, Qv], BF16, tag=f"pt{j}", name=f"pt{j}")
            for j in range(nk)]
    if tails_batched:
        pt_tails = big.tile([P, nk * TW], BF16, tag="ptt", name="ptt")
        ps_tails = psum_t.tile([P, nk * TW], F32, tag="pstl", name="ps_tails")
    elif TW > 0:
        # Per-k-tile tails, double-buffered inside a single PSUM bank
        # (2*TW*4 <= 2KB): one tile, manual ping-pong on kti parity.
        ps_tail2 = psum_t.tile([P, 2 * TW], F32, tag="pstl", name="ps_tail2")

    pso = psum_o.tile([KW, Qv], F32, tag="pso", name="pso")

    # Keep the PE continuously busy until the first input DMA lands: an
    # idle gap resets the p-state ramp and the whole kernel then runs at
    # the low PE clock.
    n_warm = int(os.environ.get("XATTN_WARMUP", "12"))
    for _ in range(n_warm):
        nc.tensor.matmul(
            pso[0:C, 0:256], lhsT=warm[:, 0:C], rhs=warm[:, 0:256],
            start=True, stop=True, skip_group_check=True,
        )

    ps_tiles = [None] * nk
    tail_tiles = [None] * nk

    def emit_s(kti):
        ps = psum_s.tile([P, M], F32, tag="pss", name=f"ps{kti}")
        ps_tiles[kti] = ps
        ktile = kt[:, kti * P:(kti + 1) * P]
        for (off, w) in m_chunks:
            nc.tensor.matmul(
                ps[:, off:off + w], lhsT=ktile, rhs=qt[:, off:off + w],
                start=True, stop=True,
            )
        if TW > 0:
            if tails_batched:
                nc.tensor.matmul(
                    ps_tails[:, kti * TW:(kti + 1) * TW],
                    lhsT=ktile, rhs=qt[:, M:Qv], start=True, stop=True,
                )
            else:
                b = (kti % 2) * TW
                pst = ps_tail2[:, b:b + TW]
                tail_tiles[kti] = pst
                nc.tensor.matmul(
                    pst, lhsT=ktile, rhs=qt[:, M:Qv],
                    start=True, stop=True,
                )

    # Tile-granular exp split: even k-tiles on ACT (exact exp), odd ones
    # on DVE (Schraudolph). One producer per P^T tile keeps every PV
    # matmul at a single semaphore wait.
    def emit_exp(kti):
        ps = ps_tiles[kti]
        if kti == nk - 1 and M > BANK_F32:
            # Last tile: split ACT/DVE at the PV chunk boundary so both
            # halves run concurrently (each PV matmul still waits on
            # exactly one producer) and the closing PV pass starts ~1us
            # sooner.
            nc.scalar.activation(
                pt_t[kti][:, 0:BANK_F32], ps[:, 0:BANK_F32],
                mybir.ActivationFunctionType.Exp, scale=SCALE,
            )
            nc.vector.tensor_scalar(
                pt_t[kti][:, BANK_F32:M].bitcast(I16), ps[:, BANK_F32:M],
                SCH_A, SCH_B, mybir.AluOpType.mult, mybir.AluOpType.add,
            )
        elif kti % 2 == 0:
            nc.scalar.activation(
                pt_t[kti][:, 0:M], ps[:],
                mybir.ActivationFunctionType.Exp, scale=SCALE,
            )
        else:
            nc.vector.tensor_scalar(
                pt_t[kti][:, 0:M].bitcast(I16), ps[:],
                SCH_A, SCH_B, mybir.AluOpType.mult, mybir.AluOpType.add,
            )
        if TW > 0 and not tails_batched:
            nc.vector.tensor_scalar(
                pt_t[kti][:, M:Qv].bitcast(I16), tail_tiles[kti],
                SCH_A, SCH_B, mybir.AluOpType.mult, mybir.AluOpType.add,
            )

    def emit_pv_main(kti):
        kvt = kv_all[:, kti * KW:(kti + 1) * KW]
        for (off, w) in o_chunks:
            if off >= M:
                continue
            nc.tensor.matmul(
                pso[:, off:off + w], lhsT=kvt, rhs=pt_t[kti][:, off:off + w],
                start=(kti == 0), stop=(kti == nk - 1),
                skip_group_check=True,
            )

    def emit_pv_tail(kti):
        if TW <= 0:
            return
        kvt = kv_all[:, kti * KW:(kti + 1) * KW]
        rhs = (pt_tails[:, kti * TW:(kti + 1) * TW] if tails_batched
               else pt_t[kti][:, M:Qv])
        nc.tensor.matmul(
            pso[:, M:Qv], lhsT=kvt, rhs=rhs,
            start=(kti == 0), stop=(kti == nk - 1),
            skip_group_check=True,
        )

    for kti in range(nk):
        emit_s(kti)
        emit_exp(kti)
        if kti >= 2:
            emit_pv_main(kti - 2)
    if tails_batched:
        # One Schraudolph pass for every k-tile's tail columns (DVE has
        # the lighter exp load; ACT carries 5 of the 9 main tiles).
        nc.vector.tensor_scalar(
            pt_tails[:].bitcast(I16), ps_tails[:],
            SCH_A, SCH_B, mybir.AluOpType.mult, mybir.AluOpType.add,
        )
    if nk >= 2:
        emit_pv_main(nk - 2)
    for kti in range(nk):
        emit_pv_tail(kti)
    emit_pv_main(nk - 1)

    # DMA cannot read PSUM: stage out^T through SBUF as bf16 (halves the
    # output DMA; host divides num/den in fp32). Split the copy between
    # ACT and DVE, DMA each half from a separate ring as soon as its
    # copy lands.
    obuf = big.tile([KW, Qv], BF16, tag="obuf", name="obuf")
    c0w = min(BANK_F32, Qv)
    nc.scalar.activation(
        obuf[:, 0:c0w], pso[:, 0:c0w],
        mybir.ActivationFunctionType.Copy,
    )
    nc.sync.dma_start(out_ap[:, 0:c0w], obuf[:, 0:c0w])
    if Qv > c0w:
        nc.vector.tensor_copy(obuf[:, c0w:Qv], pso[:, c0w:Qv])
        nc.scalar.dma_start(out_ap[:, c0w:Qv], obuf[:, c0w:Qv])


def build_program(Qp: int, Kp: int, Qv: int):
    nc = bacc.Bacc(
        trn_type="TRN2",
        target_bir_lowering=False,
        debug=False,
        num_devices=N_CORES,
    )
    nk = Kp // P
    qkt_ap = nc.dram_tensor("qkT", [C, Qp + Kp], BF16, kind="ExternalInput").ap()
    kv_ap = nc.dram_tensor("kv", [P, nk * KW], BF16, kind="ExternalInput").ap()
    out_ap = nc.dram_tensor("outT", [KW, Qv], BF16, kind="ExternalOutput").ap()
    with tile.TileContext(nc) as tc, ExitStack() as ctx:
        _emit2(ctx, tc, out_ap, qkt_ap, kv_ap, Qp, Kp, Qv)
    nc.compile()
    return nc


def shard_inputs(query, key_value, query_coors, key_value_coors):
    import ml_dtypes
    query = np.ascontiguousarray(np.asarray(query), dtype=np.float32)
    key_value = np.ascontiguousarray(np.asarray(key_value), dtype=np.float32)
    qc = np.asarray(query_coors).astype(np.int64)
    kc = np.asarray(key_value_coors).astype(np.int64)
    B = N_CORES
    ids = np.arange(B)
    qs = np.searchsorted(qc, ids, side="left")
    qe = np.searchsorted(qc, ids, side="right")
    ks = np.searchsorted(kc, ids, side="left")
    ke = np.searchsorted(kc, ids, side="right")
    qcnt, kcnt = qe - qs, ke - ks
    Qp = max(_round_up(int(qcnt.max()), P), P)
    Kp = max(_round_up(int(kcnt.max()), P), P)
    Qv = min(_round_up(int(qcnt.max()), 4), Qp)
    nk = Kp // P
    in_maps = []
    for b in range(B):
        qsh = np.zeros((Qp, C), np.float32)
        qsh[: qcnt[b]] = query[qs[b]: qe[b]]
        kvsh = np.zeros((Kp, KW), np.float32)
        kvsh[: kcnt[b], :C] = key_value[ks[b]: ke[b]]
        kvsh[: kcnt[b], C] = 1.0
        qkt = np.concatenate([qsh.T, kvsh[:, :C].T], axis=1)
        kv_il = kvsh.reshape(nk, P, KW).transpose(1, 0, 2).reshape(P, nk * KW)
        in_maps.append({
            "qkT": np.ascontiguousarray(qkt.astype(ml_dtypes.bfloat16)),
            "kv": np.ascontiguousarray(kv_il.astype(ml_dtypes.bfloat16)),
        })
    return in_maps, (qs, qe, qcnt), Qp, Kp, Qv


def kernel(query, key_value, query_coors, key_value_coors):
    in_maps, (qs, qe, qcnt), Qp, Kp, Qv = shard_inputs(
        query, key_value, query_coors, key_value_coors
    )
    nc = build_program(Qp, Kp, Qv)
    trace = bool(os.environ.get("XATTN_TRACE"))
    res = run_bass_kernel_spmd(
        nc, in_maps, list(range(N_CORES)), trace=trace,
        trace_cores=list(range(N_CORES)) if trace else None,
    )
    _LAST_RUN["exec_time_ns"] = res.exec_time_ns
    _LAST_RUN["mean_exec_time_ns"] = res.mean_exec_time_ns
    _LAST_RUN["trace"] = res.instructions_and_trace
    _LAST_RUN["results"] = res
    N1 = np.asarray(query).shape[0]
    out = np.zeros((N1, C), np.float32)
    for b in range(N_CORES):
        ot = np.asarray(res.results[b]["outT"], dtype=np.float32)  # [65, Qv]
        n = int(qcnt[b])
        num = ot[:C, :n]
        den = ot[C, :n]
        out[qs[b]: qe[b]] = (num / den[None, :]).T
    return out


# revision 34
# speedup vs baseline: 1.0400x; 1.0400x over previous
"""Per-batch (block-diagonal) cross-attention kernel for Trainium2.

Each query row attends only to key/value rows with the same batch id
(ids in [0, 8), both coor arrays sorted). Batch b -> core b: every core
runs one dense attention block of ~1k queries x ~1k keys, C=64, no
collectives.

v2 design (all sizes per core; P=128, Qv = valid queries, Kp = padded
keys, nk = Kp/128):

  Host-side layout (bf16 operands, fp32 accumulate on device):
    - qkT [64, Qp+Kp]  : [Q^T | K^T], bf16, zero-padded
    - kv  [128, nk*65] : KV rows interleaved per k-tile, col 64 = 1.0
                         on valid rows (softmax denominator trick), bf16
  Device per k-tile kti:
    - S^T[k, q] = kT_tile^T @ qT   (PE, bf16, fp32 PSUM)
    - P^T = exp(S^T / 8): split between ACT (exact exp activation) and
      DVE (Schraudolph: i16 = S*A + B, bitcast to bf16 ~= exp, ~1.5%
      per-element, cancels in softmax normalization) so neither engine
      is the wall. No max-subtraction (scores O(1) for randn inputs;
      softmax is shift-invariant).
    - out^T[c, q] += kv_tile^T @ P^T_tile  (PE, stationary = kv tile so
      ldweights is 65 cols and fully hidden; accumulated over kti in
      PSUM). Row 64 = softmax denominator.
  Tail: q columns beyond the PSUM-bank-aligned main region [0:M] are
  computed into a single batched tail PSUM tile (one exp instruction
  for all k-tiles' tails) when they fit one bank, else per-k-tile.
  Output: out^T [65, Qv] copied PSUM -> SBUF as bf16 (split across ACT
  and DVE) and DMA'd out; the host does the numerator/denominator
  divide in fp32 and the transpose back to [q, c].

  PSUM budget (8 banks): S-main 2 bufs x 2 banks + S-tail 1 bank +
  out^T 3 banks = 8.

  PV passes run interleaved two k-tiles behind S so the PE never waits
  on ACT/DVE exp, and warmup matmuls during the input-DMA head push the
  PE out of its low p-state before real work arrives.
"""

import os
from contextlib import ExitStack

import numpy as np

import concourse.bacc as bacc
import concourse.mybir as mybir
import concourse.tile as tile
from concourse.bass_utils import run_bass_kernel_spmd

N_CORES = 8
C = 64
P = 128
KW = C + 1  # kv tile width (65: values + ones column)
SCALE = 1.0 / 8.0  # 1/sqrt(C)
F32 = mybir.dt.float32
BF16 = mybir.dt.bfloat16
I16 = mybir.dt.int16

# Schraudolph exp approximation in bf16: exp(s/8) ~= bitcast_bf16(
# int16(s * A + B)). A folds the 1/sqrt(C) score scale into 2^7/ln2.
SCH_A = 184.66496736312366 / 8.0
SCH_B = 16256.0 - 7.42

BANK_F32 = 512  # fp32 elements per PSUM bank (2KB)

_LAST_RUN = {}


def _round_up(x: int, m: int) -> int:
    return -(-x // m) * m


def _emit2(ctx: ExitStack, tc: "tile.TileContext", out_ap, qkt_ap, kv_ap,
           Qp: int, Kp: int, Qv: int):
    """Emit the per-core attention program (see module docstring)."""
    nc = tc.nc
    nk = Kp // P

    M = min(Qv, 2 * BANK_F32)
    TW = Qv - M
    tails_batched = TW > 0 and nk * TW <= BANK_F32

    m_chunks = [(0, min(M, BANK_F32))]
    if M > BANK_F32:
        m_chunks.append((BANK_F32, M - BANK_F32))
    o_chunks = [(c, min(BANK_F32, Qv - c)) for c in range(0, Qv, BANK_F32)]

    big = ctx.enter_context(tc.tile_pool(name="big", bufs=1))
    psum_s = ctx.enter_context(tc.tile_pool(name="pss", bufs=2, space="PSUM"))
    psum_o = ctx.enter_context(tc.tile_pool(name="pso", bufs=1, space="PSUM"))
    if TW > 0:
        psum_t = ctx.enter_context(
            tc.tile_pool(name="pst", bufs=1, space="PSUM"))

    qkt = big.tile([C, Qp + Kp], BF16, tag="qkt", name="qkt")
    kv_all = big.tile([P, nk * KW], BF16, tag="kv_all", name="kv_all")
    warm = big.tile([C, 256], BF16, tag="warm", name="warm")

    nc.gpsimd.memset(warm[:], 0.0)

    # Ring layout: the scalar ring carries what slot 0 needs first
    # (qt-c0 then qt-tail, both small); the sync ring carries kT-head,
    # then qt-c1 (needed by S(0) chunk 2), then the rest of kT (first
    # needed at slot 2). kv goes via gpsimd SWDGE in parallel.
    c0w = min(BANK_F32, M)
    nc.sync.dma_start(qkt[:, Qp:Qp + 2 * P], qkt_ap[:, Qp:Qp + 2 * P])
    nc.scalar.dma_start(qkt[:, 0:c0w], qkt_ap[:, 0:c0w])
    if M > c0w:
        nc.sync.dma_start(qkt[:, c0w:M], qkt_ap[:, c0w:M])
    if M < Qp:
        nc.scalar.dma_start(qkt[:, M:Qp], qkt_ap[:, M:Qp])
    nc.sync.dma_start(qkt[:, Qp + 2 * P:Qp + Kp], qkt_ap[:, Qp + 2 * P:Qp + Kp])
    nc.gpsimd.dma_start(kv_all[:], kv_ap[:, :])

    qt = qkt[:, 0:Qp]
    kt = qkt[:, Qp:Qp + Kp]

    pt_t = [big.tile([P, Qv], BF16, tag=f"pt{j}", name=f"pt{j}")
            for j in range(nk)]
    if tails_batched:
        pt_tails = big.tile([P, nk * TW], BF16, tag="ptt", name="ptt")
        ps_tails = psum_t.tile([P, nk * TW], F32, tag="pstl", name="ps_tails")
    elif TW > 0:
        # Per-k-tile tails, double-buffered inside a single PSUM bank
        # (2*TW*4 <= 2KB): one tile, manual ping-pong on kti parity.
        ps_tail2 = psum_t.tile([P, 2 * TW], F32, tag="pstl", name="ps_tail2")

    pso = psum_o.tile([KW, Qv], F32, tag="pso", name="pso")

    # Keep the PE continuously busy until the first input DMA lands: an
    # idle gap resets the p-state ramp and the whole kernel then runs at
    # the low PE clock.
    n_warm = int(os.environ.get("XATTN_WARMUP", "12"))
    for _ in range(n_warm):
        nc.tensor.matmul(
            pso[0:C, 0:256], lhsT=warm[:, 0:C], rhs=warm[:, 0:256],
            start=True, stop=True, skip_group_check=True,
        )

    ps_tiles = [None] * nk
    tail_tiles = [None] * nk

    def emit_s(kti):
        ps = psum_s.tile([P, M], F32, tag="pss", name=f"ps{kti}")
        ps_tiles[kti] = ps
        ktile = kt[:, kti * P:(kti + 1) * P]
        for (off, w) in m_chunks:
            nc.tensor.matmul(
                ps[:, off:off + w], lhsT=ktile, rhs=qt[:, off:off + w],
                start=True, stop=True,
            )
        if TW > 0:
            if tails_batched:
                nc.tensor.matmul(
                    ps_tails[:, kti * TW:(kti + 1) * TW],
                    lhsT=ktile, rhs=qt[:, M:Qv], start=True, stop=True,
                )
            else:
                b = (kti % 2) * TW
                pst = ps_tail2[:, b:b + TW]
                tail_tiles[kti] = pst
                nc.tensor.matmul(
                    pst, lhsT=ktile, rhs=qt[:, M:Qv],
                    start=True, stop=True,
                )

    # Tile-granular exp split: even k-tiles on ACT (exact exp), odd ones
    # on DVE (Schraudolph). One producer per P^T tile keeps every PV
    # matmul at a single semaphore wait.
    def emit_exp(kti):
        ps = ps_tiles[kti]
        if kti == nk - 1 and M > BANK_F32:
            # Last tile: split ACT/DVE at the PV chunk boundary so both
            # halves run concurrently (each PV matmul still waits on
            # exactly one producer) and the closing PV pass starts ~1us
            # sooner.
            nc.scalar.activation(
                pt_t[kti][:, 0:BANK_F32], ps[:, 0:BANK_F32],
                mybir.ActivationFunctionType.Exp, scale=SCALE,
            )
            nc.vector.tensor_scalar(
                pt_t[kti][:, BANK_F32:M].bitcast(I16), ps[:, BANK_F32:M],
                SCH_A, SCH_B, mybir.AluOpType.mult, mybir.AluOpType.add,
            )
        elif kti % 2 == 0:
            nc.scalar.activation(
                pt_t[kti][:, 0:M], ps[:],
                mybir.ActivationFunctionType.Exp, scale=SCALE,
            )
        else:
            nc.vector.tensor_scalar(
                pt_t[kti][:, 0:M].bitcast(I16), ps[:],
                SCH_A, SCH_B, mybir.AluOpType.mult, mybir.AluOpType.add,
            )
        if TW > 0 and not tails_batched:
            nc.vector.tensor_scalar(
                pt_t[kti][:, M:Qv].bitcast(I16), tail_tiles[kti],
                SCH_A, SCH_B, mybir.AluOpType.mult, mybir.AluOpType.add,
            )

    def emit_pv_main(kti):
        kvt = kv_all[:, kti * KW:(kti + 1) * KW]
        for (off, w) in o_chunks:
            if off >= M:
                continue
            nc.tensor.matmul(
                pso[:, off:off + w], lhsT=kvt, rhs=pt_t[kti][:, off:off + w],
                start=(kti == 0), stop=(kti == nk - 1),
                skip_group_check=True,
            )

    def emit_pv_tail(kti):
        if TW <= 0:
            return
        kvt = kv_all[:, kti * KW:(kti + 1) * KW]
        rhs = (pt_tails[:, kti * TW:(kti + 1) * TW] if tails_batched
               else pt_t[kti][:, M:Qv])
        nc.tensor.matmul(
            pso[:, M:Qv], lhsT=kvt, rhs=rhs,
            start=(kti == 0), stop=(kti == nk - 1),
            skip_group_check=True,
        )

    for kti in range(nk):
        emit_s(kti)
        emit_exp(kti)
        if kti >= 2:
            emit_pv_main(kti - 2)
    if tails_batched:
        # One Schraudolph pass for every k-tile's tail columns (DVE has
        # the lighter exp load; ACT carries 5 of the 9 main tiles).
        nc.vector.tensor_scalar(
            pt_tails[:].bitcast(I16), ps_tails[:],
            SCH_A, SCH_B, mybir.AluOpType.mult, mybir.AluOpType.add,
        )
    if nk >= 2:
        emit_pv_main(nk - 2)
    for kti in range(nk):
        emit_pv_tail(kti)
    emit_pv_main(nk - 1)

    # DMA cannot read PSUM: stage out^T through SBUF as bf16 (halves the
    # output DMA; host divides num/den in fp32). Split the copy between
    # ACT and DVE, DMA each half from a separate ring as soon as its
    # copy lands.
    obuf = big.tile([KW, Qv], BF16, tag="obuf", name="obuf")
    c0w = min(BANK_F32, Qv)
    nc.scalar.activation(
        obuf[:, 0:c0w], pso[:, 0:c0w],
        mybir.ActivationFunctionType.Copy,
    )
    nc.sync.dma_start(out_ap[:, 0:c0w], obuf[:, 0:c0w])
    if Qv > c0w:
        nc.vector.tensor_copy(obuf[:, c0w:Qv], pso[:, c0w:Qv])
        nc.scalar.dma_start(out_ap[:, c0w:Qv], obuf[:, c0w:Qv])


def build_program(Qp: int, Kp: int, Qv: int):
    nc = bacc.Bacc(
        trn_type="TRN2",
        target_bir_lowering=False,
        debug=False,
        num_devices=N_CORES,
    )
    nk = Kp // P
    qkt_ap = nc.dram_tensor("qkT", [C, Qp + Kp], BF16, kind="ExternalInput").ap()
    kv_ap = nc.dram_tensor("kv", [P, nk * KW], BF16, kind="ExternalInput").ap()
    out_ap = nc.dram_tensor("outT", [KW, Qv], BF16, kind="ExternalOutput").ap()
    with tile.TileContext(nc) as tc, ExitStack() as ctx:
        _emit2(ctx, tc, out_ap, qkt_ap, kv_ap, Qp, Kp, Qv)
    nc.compile()
    return nc


def shard_inputs(query, key_value, query_coors, key_value_coors):
    import ml_dtypes
    query = np.ascontiguousarray(np.asarray(query), dtype=np.float32)
    key_value = np.ascontiguousarray(np.asarray(key_value), dtype=np.float32)
    qc = np.asarray(query_coors).astype(np.int64)
    kc = np.asarray(key_value_coors).astype(np.int64)
    B = N_CORES
    ids = np.arange(B)
    qs = np.searchsorted(qc, ids, side="left")
    qe = np.searchsorted(qc, ids, side="right")
    ks = np.searchsorted(kc, ids, side="left")
    ke = np.searchsorted(kc, ids, side="right")
    qcnt, kcnt = qe - qs, ke - ks
    Qp = max(_round_up(int(qcnt.max()), P), P)
    Kp = max(_round_up(int(kcnt.max()), P), P)
    Qv = min(_round_up(int(qcnt.max()), 4), Qp)
    nk = Kp // P
    in_maps = []
    for b in range(B):
        qsh = np.zeros((Qp, C), np.float32)
        qsh[: qcnt[b]] = query[qs[b]: qe[b]]
        kvsh = np.zeros((Kp, KW), np.float32)
        kvsh[: kcnt[b], :C] = key_value[ks[b]: ke[b]]
        kvsh[: kcnt[b], C] = 1.0
        qkt = np.concatenate([qsh.T, kvsh[:, :C].T], axis=1)
        kv_il = kvsh.reshape(nk, P, KW).transpose(1, 0, 2).reshape(P, nk * KW)
        in_maps.append({
            "qkT": np.ascontiguousarray(qkt.astype(ml_dtypes.bfloat16)),
            "kv": np.ascontiguousarray(kv_il.astype(ml_dtypes.bfloat16)),
        })
    return in_maps, (qs, qe, qcnt), Qp, Kp, Qv


def kernel(query, key_value, query_coors, key_value_coors):
    in_maps, (qs, qe, qcnt), Qp, Kp, Qv = shard_inputs(
        query, key_value, query_coors, key_value_coors
    )
    nc = build_program(Qp, Kp, Qv)
    trace = bool(os.environ.get("XATTN_TRACE"))
    res = run_bass_kernel_spmd(
        nc, in_maps, list(range(N_CORES)), trace=trace,
        trace_cores=list(range(N_CORES)) if trace else None,
    )
    _LAST_RUN["exec_time_ns"] = res.exec_time_ns
    _LAST_RUN["mean_exec_time_ns"] = res.mean_exec_time_ns
    _LAST_RUN["trace"] = res.instructions_and_trace
    _LAST_RUN["results"] = res
    N1 = np.asarray(query).shape[0]
    out = np.zeros((N1, C), np.float32)
    for b in range(N_CORES):
        ot = np.asarray(res.results[b]["outT"], dtype=np.float32)  # [65, Qv]
        n = int(qcnt[b])
        num = ot[:C, :n]
        den = ot[C, :n]
        out[qs[b]: qe[b]] = (num / den[None, :]).T
    return out
